# revision 2
# baseline (speedup 1.0000x reference)
import sys

sys.path.insert(0, "/opt/trn_rl_repo")

import numpy as np

G, E, N, H = 8, 8192, 512, 32
NP1 = N + 1          # 513
T = N * N            # 262144 tokens per graph
NG = 512             # token groups of 512
NQ = NG // 4         # 128 quads
LAG = 8              # stage1 -> stage2 lag in groups (multiple of 4)
RL = LAG // 2 + 4    # hh ring size in pairs


# ----------------------------------------------------------------- device code
def build(nc, outs, ins):
    from contextlib import ExitStack

    import concourse.tile as tile
    from concourse import mybir

    f32 = mybir.dt.float32
    fp16 = mybir.dt.float16
    Relu = mybir.ActivationFunctionType.Relu
    Alu = mybir.AluOpType

    out2 = outs["out"]               # [128, T//4] f32; row = (g%4)*32+h
    xln = ins["xln"]                 # [112, T] fp16 (X rows 0:56, xi rows 56:112)
    w1a = ins["w1a"]                 # [112, 64] fp16 = [A1; A1]
    w1b = ins["w1b"]                 # [56, 64] fp16 = a1 (lo residue of w1)
    w2a = ins["w2a"]                 # [128, 32] fp16 = [A2; A2]
    w2b = ins["w2b"]                 # [128, 32] fp16 = [a2; a2]

    with tile.TileContext(nc) as tc, ExitStack() as ctx:
        cst = ctx.enter_context(tc.tile_pool(name="cst", bufs=1))
        w1a_s = cst.tile([112, 64], fp16)
        nc.sync.dma_start(out=w1a_s[:], in_=w1a[:])
        w1b_s = cst.tile([56, 64], fp16)
        nc.sync.dma_start(out=w1b_s[:], in_=w1b[:])
        w2a_s = cst.tile([128, 32], fp16)
        nc.sync.dma_start(out=w2a_s[:], in_=w2a[:])
        w2b_s = cst.tile([128, 32], fp16)
        nc.sync.dma_start(out=w2b_s[:], in_=w2b[:])

        # x input staged per quad (4 groups) via SWDGE striping
        xin = ctx.enter_context(tc.tile_pool(name="xin", bufs=5))
        xq_tiles = {}

        def load_quad(q):
            if q >= NQ or q in xq_tiles:
                return
            t = xin.tile([112, 2048], fp16, tag="xq")
            nc.gpsimd.dma_start(out=t[:], in_=xln[:, q * 2048:(q + 1) * 2048])
            xq_tiles[q] = t

        for q in range(4):
            load_quad(q)

        hhp = ctx.enter_context(tc.tile_pool(name="hhp", bufs=RL))
        osp = ctx.enter_context(tc.tile_pool(name="osp", bufs=3))
        ps1 = ctx.enter_context(tc.tile_pool(name="ps1", bufs=3, space="PSUM"))
        ps2 = ctx.enter_context(tc.tile_pool(name="ps2", bufs=2, space="PSUM"))

        hh_ring = [None] * RL
        hps = None
        ops = None

        for g in range(NG + LAG):
            if g < NG:
                q, j4 = g // 4, g % 4
                if j4 == 0:
                    load_quad(q + 4)
                xq = xq_tiles[q]
                pj = g % 2
                if pj == 0:
                    hps = ps1.tile([64, 1024], f32, tag="hps")
                dst = hps[0:64, pj * 512:(pj + 1) * 512]
                nc.tensor.matmul(out=dst, lhsT=w1a_s[:],
                                 rhs=xq[0:112, j4 * 512:(j4 + 1) * 512],
                                 start=True, stop=False)
                nc.tensor.matmul(out=dst, lhsT=w1b_s[:],
                                 rhs=xq[0:56, j4 * 512:(j4 + 1) * 512],
                                 start=False, stop=True)
                if pj == 1:
                    P = g // 2
                    hh = hhp.tile([128, 1024], fp16, tag="hh")
                    # H = fp16(relu(h)); eta = fp16(relu(h) - H)
                    nc.scalar.activation(out=hh[0:64, :], in_=hps[0:64, :],
                                         func=Relu)
                    nc.vector.scalar_tensor_tensor(
                        out=hh[64:128, :], in0=hps[0:64, :], scalar=0.0,
                        in1=hh[0:64, :], op0=Alu.max, op1=Alu.subtract,
                    )
                    hh_ring[P % RL] = hh
            if g >= LAG:
                g2 = g - LAG
                j4 = g2 % 4
                if j4 == 0:
                    ops = ps2.tile([128, 512], f32, tag="ops")
                P2, pj2 = g2 // 2, g2 % 2
                hh2 = hh_ring[P2 % RL]
                rhs2 = hh2[0:128, pj2 * 512:(pj2 + 1) * 512]
                q0 = j4 * 32
                nc.tensor.matmul(out=ops[q0:q0 + 32, :], lhsT=w2a_s[:],
                                 rhs=rhs2, start=True, stop=False,
                                 tile_position=(0, q0))
                nc.tensor.matmul(out=ops[q0:q0 + 32, :], lhsT=w2b_s[:],
                                 rhs=rhs2, start=False, stop=True,
                                 tile_position=(0, q0))
                if j4 == 3:
                    qq = g2 // 4
                    osb = osp.tile([128, 512], f32, tag="osb")
                    if qq % 2 == 0:
                        nc.scalar.copy(out=osb[:], in_=ops[:])
                    else:
                        nc.vector.tensor_copy(out=osb[:], in_=ops[:])
                    nc.sync.dma_start(out=out2[:, qq * 512:(qq + 1) * 512],
                                      in_=osb[:])


# ----------------------------------------------------------------- host prep
def _split16(x):
    hi = x.astype(np.float16)
    lo = (x - hi.astype(np.float32)).astype(np.float16)
    return hi, lo


def prep_weights(inputs):
    w1 = np.zeros((56, 64), np.float32)
    w1[0:28, 0:32] = np.asarray(inputs["ang_w1"], np.float32)
    w1[28:56, 32:64] = np.asarray(inputs["md_w1"], np.float32)
    b1 = np.concatenate([np.asarray(inputs["ang_b1"]),
                         np.asarray(inputs["md_b1"])]).astype(np.float32)
    assert not np.any(b1), "kernel assumes zero hidden bias"
    w2 = np.concatenate([np.asarray(inputs["ang_w2"], np.float32),
                         np.asarray(inputs["md_w2"], np.float32)], 0)
    A1, a1 = _split16(w1)
    A2, a2 = _split16(w2)
    w1a = np.concatenate([A1, A1], 0)                       # [112, 64]
    w2a = np.concatenate([A2, A2], 0)                       # [128, 32]
    w2b = np.concatenate([a2, a2], 0)                       # [128, 32]
    b2 = (np.asarray(inputs["ang_b2"]) + np.asarray(inputs["md_b2"]))
    return w1a, a1, w2a, w2b, b2.astype(np.float32)


def prep_x(g, inputs):
    ang = np.asarray(inputs["angle"][g], np.float32).reshape(T, 28)
    dst = np.asarray(inputs["dists"][g], np.float32).reshape(T, 28)
    x = np.concatenate([ang, dst], 1).T                     # [56, T]
    X, xi = _split16(np.ascontiguousarray(x))
    return np.concatenate([X, xi], 0)                       # [112, T]


def edge_bias_host(g, inputs, full):
    """Exact f32 edge-embedding scatter, matching the reference."""
    ef = np.asarray(inputs["edge_feat"][g], np.float32)
    ei = np.asarray(inputs["edge_index"][g]).astype(np.int64)
    mask = np.asarray(inputs["edge_mask"][g]).astype(bool)
    nlig = max(int(inputs["num_ligand_atoms"][g]), 1)

    t0 = ef[:, 0].astype(np.int32)
    t1 = ef[:, 1].astype(np.int32)
    t2 = ef[:, 2].astype(np.int32)
    d = ef[:, 3:4]                                          # [E,1]
    src, tgt = ei[0], ei[1]
    src_l = (src > 0) & (src < nlig)
    tgt_l = (tgt > 0) & (tgt < nlig)

    dw1 = np.asarray(inputs["dist_w1"], np.float32)
    db1 = np.asarray(inputs["dist_b1"], np.float32)
    dw2 = np.asarray(inputs["dist_w2"], np.float32)
    db2 = np.asarray(inputs["dist_b2"], np.float32)
    demb = np.maximum(d @ dw1 + db1, 0.0) @ dw2 + db2       # [E, 32]

    sidx = np.clip(t0 * 4 + t1 * 2 + t2, 0, 19)
    structural = np.asarray(inputs["struct_emb"], np.float32)[sidx]
    pidx = np.clip(t1, 0, 14)
    both_l = src_l & tgt_l
    both_p = (~src_l) & (~tgt_l)
    plip = np.where(
        both_l[:, None], np.asarray(inputs["plip_lig"], np.float32)[pidx],
        np.where(both_p[:, None], np.asarray(inputs["plip_prot"], np.float32)[pidx],
                 np.asarray(inputs["plip_inter"], np.float32)[pidx]))
    emb = np.where((t0 <= 1)[:, None], structural,
                   np.where((t0 == 5)[:, None], plip, 0.0)) + demb
    emb = emb * mask[:, None].astype(np.float32)

    flat = full.reshape(-1)
    cell = ((src + 1) * NP1 + (tgt + 1)).astype(np.int64)   # [E]
    idx = (np.arange(H, dtype=np.int64) * (NP1 * NP1))[None, :] + cell[:, None]
    np.add.at(flat, idx.ravel(), emb.astype(np.float32).ravel())


_IN_SPECS = [
    ("xln", (112, T), "float16"),
    ("w1a", (112, 64), "float16"),
    ("w1b", (56, 64), "float16"),
    ("w2a", (128, 32), "float16"),
    ("w2b", (128, 32), "float16"),
]


def _build_nc():
    from concourse import bacc, mybir

    nc = bacc.Bacc(
        "TRN2",
        target_bir_lowering=False,
        debug=False,
        enable_asserts=False,
        num_devices=8,
    )
    ins = {}
    for name, shape, dt in _IN_SPECS:
        h = nc.dram_tensor(name, list(shape), getattr(mybir.dt, dt),
                           kind="ExternalInput")
        ins[name] = h[:]
    out_h = nc.dram_tensor("out", [128, T // 4], mybir.dt.float32,
                           kind="ExternalOutput")
    build(nc, {"out": out_h[:]}, ins)
    nc.compile()
    return nc


def kernel(_trace=False, **inputs):
    from concourse.bass_utils import run_bass_kernel_spmd

    w1a, w1b, w2a, w2b, b2 = prep_weights(inputs)
    in_maps = []
    for g in range(G):
        in_maps.append(dict(xln=prep_x(g, inputs), w1a=w1a, w1b=w1b,
                            w2a=w2a, w2b=w2b))

    nc = _build_nc()
    res = run_bass_kernel_spmd(nc, in_maps, core_ids=list(range(G)),
                               trace=_trace)
    if _trace:
        print("HW exec time:", res.exec_time_ns, "ns  (mean:",
              res.mean_exec_time_ns, "ns, slowest core:",
              res.max_exec_time_core_id, ")")
        if res.instructions_and_trace:
            print("trace:", res.instructions_and_trace[1])

    attn = np.asarray(inputs["attn_bias"], np.float32)      # [G, 513, 513]
    virt = np.asarray(inputs["virt"], np.float32).reshape(H)
    outs = []
    for g, r in enumerate(res.results):
        # device rows: (g%4)*32 + h; cols: (g//4)*512 + c
        dev = r["out"].reshape(4, 32, NQ, 512).transpose(1, 2, 0, 3)
        dev = dev.reshape(32, T).reshape(32, N, N)          # [H, i, j]
        full = np.empty((H, NP1, NP1), np.float32)
        full[:, 1:, 1:] = dev + b2[:, None, None] + attn[g][None, 1:, 1:]
        full[:, 1:, 0] = attn[g][None, 1:, 0] + virt[:, None]
        full[:, 0, :] = attn[g][None, 0, :] + virt[:, None]
        edge_bias_host(g, inputs, full)
        outs.append(full)
    return np.stack(outs).astype(np.float32)


# revision 6
# speedup vs baseline: 1.8953x; 1.8953x over previous
import sys

sys.path.insert(0, "/opt/trn_rl_repo")

import numpy as np

G, E, N, H = 8, 8192, 512, 32
NP1 = N + 1          # 513
T = N * N            # 262144 tokens per graph
NG = 512             # token groups of 512
NQ = NG // 4         # 128 quads
LAG = 8              # stage1 -> stage2 lag in groups (multiple of 4)
RL = LAG // 2 + 4    # hh ring size in pairs


# ----------------------------------------------------------------- device code
def build(nc, outs, ins):
    from contextlib import ExitStack

    import concourse.tile as tile
    from concourse import mybir

    f32 = mybir.dt.float32
    fp16 = mybir.dt.float16
    Relu = mybir.ActivationFunctionType.Relu
    Alu = mybir.AluOpType

    out2 = outs["out"]               # [128, T//4] f32; row = (g%4)*32+h
    xln = ins["xln"]                 # [112, T] fp16 (X rows 0:56, xi rows 56:112)
    w1a = ins["w1a"]                 # [112, 64] fp16 = [A1; A1]
    w1b = ins["w1b"]                 # [112, 64] fp16 = [a1; a1]
    w2a = ins["w2a"]                 # [128, 32] fp16 = [A2; A2]
    w2b = ins["w2b"]                 # [128, 32] fp16 = [a2; a2]

    with tile.TileContext(nc) as tc, ExitStack() as ctx:
        cst = ctx.enter_context(tc.tile_pool(name="cst", bufs=1))
        w1a_s = cst.tile([112, 64], fp16)
        nc.sync.dma_start(out=w1a_s[:], in_=w1a[:])
        w1b_s = cst.tile([112, 64], fp16)
        nc.sync.dma_start(out=w1b_s[:], in_=w1b[:])
        w2a_s = cst.tile([128, 32], fp16)
        nc.sync.dma_start(out=w2a_s[:], in_=w2a[:])
        w2b_s = cst.tile([128, 32], fp16)
        nc.sync.dma_start(out=w2b_s[:], in_=w2b[:])

        # x input staged per quad (4 groups) via SWDGE striping
        xin = ctx.enter_context(tc.tile_pool(name="xin", bufs=5))
        xq_tiles = {}

        def load_quad(q):
            if q >= NQ or q in xq_tiles:
                return
            t = xin.tile([112, 2048], fp16, tag="xq")
            nc.gpsimd.dma_start(out=t[:], in_=xln[:, q * 2048:(q + 1) * 2048])
            xq_tiles[q] = t

        for q in range(4):
            load_quad(q)

        hhp = ctx.enter_context(tc.tile_pool(name="hhp", bufs=RL))
        osp = ctx.enter_context(tc.tile_pool(name="osp", bufs=3))
        ps1 = ctx.enter_context(tc.tile_pool(name="ps1", bufs=3, space="PSUM"))
        ps2 = ctx.enter_context(tc.tile_pool(name="ps2", bufs=2, space="PSUM"))

        hh_ring = [None] * RL
        hps_ring = [None] * 3
        hps = None
        ops = None

        def eta_op(P):
            # eta(P) = fp16(relu(h) - H); emitted one pair late so it runs
            # on DVE concurrently with the next pair's H-op on Act
            hh_prev = hh_ring[P % RL]
            nc.vector.scalar_tensor_tensor(
                out=hh_prev[64:128, :], in0=hps_ring[P % 3][0:64, :],
                scalar=0.0, in1=hh_prev[0:64, :],
                op0=Alu.max, op1=Alu.subtract,
            )

        for g in range(NG + LAG):
            if g < NG:
                q, j4 = g // 4, g % 4
                if j4 == 0:
                    load_quad(q + 4)
                xq = xq_tiles[q]
                pj = g % 2
                if pj == 0:
                    hps = ps1.tile([64, 1024], f32, tag="hps")
                    hps_ring[(g // 2) % 3] = hps
                # same-K back-to-back matmuls: main then lo-residue pass
                dst = hps[0:64, pj * 512:(pj + 1) * 512]
                rhs = xq[0:112, j4 * 512:(j4 + 1) * 512]
                nc.tensor.matmul(out=dst, lhsT=w1a_s[:], rhs=rhs,
                                 start=True, stop=False)
                nc.tensor.matmul(out=dst, lhsT=w1b_s[:], rhs=rhs,
                                 start=False, stop=True)
                if pj == 1:
                    P = g // 2
                    hh = hhp.tile([128, 1024], fp16, tag="hh")
                    # H = fp16(relu(h))
                    nc.scalar.activation(out=hh[0:64, :], in_=hps[0:64, :],
                                         func=Relu)
                    hh_ring[P % RL] = hh
                    if P >= 1:
                        eta_op(P - 1)
                    if P == NG // 2 - 1:
                        eta_op(P)
            if g >= LAG:
                g2 = g - LAG
                j4 = g2 % 4
                if j4 == 0:
                    ops = ps2.tile([128, 512], f32, tag="ops")
                P2, pj2 = g2 // 2, g2 % 2
                hh2 = hh_ring[P2 % RL]
                rhs2 = hh2[0:128, pj2 * 512:(pj2 + 1) * 512]
                q0 = j4 * 32
                nc.tensor.matmul(out=ops[q0:q0 + 32, :], lhsT=w2a_s[:],
                                 rhs=rhs2, start=True, stop=False,
                                 tile_position=(0, q0))
                nc.tensor.matmul(out=ops[q0:q0 + 32, :], lhsT=w2b_s[:],
                                 rhs=rhs2, start=False, stop=True,
                                 tile_position=(0, q0))
                if j4 == 3:
                    qq = g2 // 4
                    osb = osp.tile([128, 512], f32, tag="osb")
                    if qq % 2 == 0:
                        nc.scalar.copy(out=osb[:], in_=ops[:])
                    else:
                        nc.vector.tensor_copy(out=osb[:], in_=ops[:])
                    nc.sync.dma_start(out=out2[:, qq * 512:(qq + 1) * 512],
                                      in_=osb[:])


# ----------------------------------------------------------------- host prep
def _split16(x):
    hi = x.astype(np.float16)
    lo = (x - hi.astype(np.float32)).astype(np.float16)
    return hi, lo


def prep_weights(inputs):
    w1 = np.zeros((56, 64), np.float32)
    w1[0:28, 0:32] = np.asarray(inputs["ang_w1"], np.float32)
    w1[28:56, 32:64] = np.asarray(inputs["md_w1"], np.float32)
    b1 = np.concatenate([np.asarray(inputs["ang_b1"]),
                         np.asarray(inputs["md_b1"])]).astype(np.float32)
    assert not np.any(b1), "kernel assumes zero hidden bias"
    w2 = np.concatenate([np.asarray(inputs["ang_w2"], np.float32),
                         np.asarray(inputs["md_w2"], np.float32)], 0)
    A1, a1 = _split16(w1)
    A2, a2 = _split16(w2)
    w1a = np.concatenate([A1, A1], 0)                       # [112, 64]
    w1b = np.concatenate([a1, a1], 0)                       # [112, 64]
    w2a = np.concatenate([A2, A2], 0)                       # [128, 32]
    w2b = np.concatenate([a2, a2], 0)                       # [128, 32]
    b2 = (np.asarray(inputs["ang_b2"]) + np.asarray(inputs["md_b2"]))
    return w1a, w1b, w2a, w2b, b2.astype(np.float32)


def prep_x(g, inputs):
    ang = np.asarray(inputs["angle"][g], np.float32).reshape(T, 28)
    dst = np.asarray(inputs["dists"][g], np.float32).reshape(T, 28)
    x = np.concatenate([ang, dst], 1).T                     # [56, T]
    X, xi = _split16(np.ascontiguousarray(x))
    return np.concatenate([X, xi], 0)                       # [112, T]


def edge_bias_host(g, inputs, full):
    """Exact f32 edge-embedding scatter, matching the reference."""
    ef = np.asarray(inputs["edge_feat"][g], np.float32)
    ei = np.asarray(inputs["edge_index"][g]).astype(np.int64)
    mask = np.asarray(inputs["edge_mask"][g]).astype(bool)
    nlig = max(int(inputs["num_ligand_atoms"][g]), 1)

    t0 = ef[:, 0].astype(np.int32)
    t1 = ef[:, 1].astype(np.int32)
    t2 = ef[:, 2].astype(np.int32)
    d = ef[:, 3:4]                                          # [E,1]
    src, tgt = ei[0], ei[1]
    src_l = (src > 0) & (src < nlig)
    tgt_l = (tgt > 0) & (tgt < nlig)

    dw1 = np.asarray(inputs["dist_w1"], np.float32)
    db1 = np.asarray(inputs["dist_b1"], np.float32)
    dw2 = np.asarray(inputs["dist_w2"], np.float32)
    db2 = np.asarray(inputs["dist_b2"], np.float32)
    demb = np.maximum(d @ dw1 + db1, 0.0) @ dw2 + db2       # [E, 32]

    sidx = np.clip(t0 * 4 + t1 * 2 + t2, 0, 19)
    structural = np.asarray(inputs["struct_emb"], np.float32)[sidx]
    pidx = np.clip(t1, 0, 14)
    both_l = src_l & tgt_l
    both_p = (~src_l) & (~tgt_l)
    plip = np.where(
        both_l[:, None], np.asarray(inputs["plip_lig"], np.float32)[pidx],
        np.where(both_p[:, None], np.asarray(inputs["plip_prot"], np.float32)[pidx],
                 np.asarray(inputs["plip_inter"], np.float32)[pidx]))
    emb = np.where((t0 <= 1)[:, None], structural,
                   np.where((t0 == 5)[:, None], plip, 0.0)) + demb
    emb = emb * mask[:, None].astype(np.float32)

    flat = full.reshape(-1)
    cell = ((src + 1) * NP1 + (tgt + 1)).astype(np.int64)   # [E]
    idx = (np.arange(H, dtype=np.int64) * (NP1 * NP1))[None, :] + cell[:, None]
    np.add.at(flat, idx.ravel(), emb.astype(np.float32).ravel())


_IN_SPECS = [
    ("xln", (112, T), "float16"),
    ("w1a", (112, 64), "float16"),
    ("w1b", (112, 64), "float16"),
    ("w2a", (128, 32), "float16"),
    ("w2b", (128, 32), "float16"),
]


def _build_nc():
    from concourse import bacc, mybir

    nc = bacc.Bacc(
        "TRN2",
        target_bir_lowering=False,
        debug=False,
        enable_asserts=False,
        num_devices=8,
    )
    ins = {}
    for name, shape, dt in _IN_SPECS:
        h = nc.dram_tensor(name, list(shape), getattr(mybir.dt, dt),
                           kind="ExternalInput")
        ins[name] = h[:]
    out_h = nc.dram_tensor("out", [128, T // 4], mybir.dt.float32,
                           kind="ExternalOutput")
    build(nc, {"out": out_h[:]}, ins)
    nc.compile()
    return nc


def kernel(_trace=False, **inputs):
    from concourse.bass_utils import run_bass_kernel_spmd

    w1a, w1b, w2a, w2b, b2 = prep_weights(inputs)
    in_maps = []
    for g in range(G):
        in_maps.append(dict(xln=prep_x(g, inputs), w1a=w1a, w1b=w1b,
                            w2a=w2a, w2b=w2b))

    nc = _build_nc()
    res = run_bass_kernel_spmd(nc, in_maps, core_ids=list(range(G)),
                               trace=_trace)
    if _trace:
        print("HW exec time:", res.exec_time_ns, "ns  (mean:",
              res.mean_exec_time_ns, "ns, slowest core:",
              res.max_exec_time_core_id, ")")
        if res.instructions_and_trace:
            print("trace:", res.instructions_and_trace[1])

    attn = np.asarray(inputs["attn_bias"], np.float32)      # [G, 513, 513]
    virt = np.asarray(inputs["virt"], np.float32).reshape(H)
    outs = []
    for g, r in enumerate(res.results):
        # device rows: (g%4)*32 + h; cols: (g//4)*512 + c
        dev = r["out"].reshape(4, 32, NQ, 512).transpose(1, 2, 0, 3)
        dev = dev.reshape(32, T).reshape(32, N, N)          # [H, i, j]
        full = np.empty((H, NP1, NP1), np.float32)
        full[:, 1:, 1:] = dev + b2[:, None, None] + attn[g][None, 1:, 1:]
        full[:, 1:, 0] = attn[g][None, 1:, 0] + virt[:, None]
        full[:, 0, :] = attn[g][None, 0, :] + virt[:, None]
        edge_bias_host(g, inputs, full)
        outs.append(full)
    return np.stack(outs).astype(np.float32)


# revision 7
# speedup vs baseline: 2.6368x; 1.3912x over previous
import sys

sys.path.insert(0, "/opt/trn_rl_repo")

import numpy as np

G, E, N, H = 8, 8192, 512, 32
NP1 = N + 1          # 513
T = N * N            # 262144 tokens per graph
NG = 512             # token groups of 512
NQ = NG // 4         # 128 quads
LAG = 8              # stage1 -> stage2 lag in groups (multiple of 4)
RL = LAG // 2 + 4    # hh ring size in pairs


# ----------------------------------------------------------------- device code
def build(nc, outs, ins):
    from contextlib import ExitStack

    import concourse.tile as tile
    from concourse import mybir

    f32 = mybir.dt.float32
    fp16 = mybir.dt.float16
    Relu = mybir.ActivationFunctionType.Relu
    Alu = mybir.AluOpType

    out2 = outs["out"]               # [128, T//4] f32; row = (g%4)*32+h
    xln = ins["xln"]                 # [112, T] fp16 (X rows 0:56, xi rows 56:112)
    w1a = ins["w1a"]                 # [112, 64] fp16 = [A1; A1]
    w1b = ins["w1b"]                 # [112, 64] fp16 = [a1; a1]
    w2a = ins["w2a"]                 # [128, 32] fp16 = [A2; A2]
    w2b = ins["w2b"]                 # [128, 32] fp16 = [a2; a2]

    with tile.TileContext(nc) as tc, ExitStack() as ctx:
        cst = ctx.enter_context(tc.tile_pool(name="cst", bufs=1))
        w1a_s = cst.tile([112, 64], fp16)
        nc.sync.dma_start(out=w1a_s[:], in_=w1a[:])
        w1b_s = cst.tile([112, 64], fp16)
        nc.sync.dma_start(out=w1b_s[:], in_=w1b[:])
        w2a_s = cst.tile([128, 32], fp16)
        nc.sync.dma_start(out=w2a_s[:], in_=w2a[:])
        w2b_s = cst.tile([128, 32], fp16)
        nc.sync.dma_start(out=w2b_s[:], in_=w2b[:])

        # x input staged per quad (4 groups) via SWDGE striping
        xin = ctx.enter_context(tc.tile_pool(name="xin", bufs=5))
        xq_tiles = {}

        def load_quad(q):
            if q >= NQ or q in xq_tiles:
                return
            t = xin.tile([112, 2048], fp16, tag="xq")
            nc.gpsimd.dma_start(out=t[:], in_=xln[:, q * 2048:(q + 1) * 2048])
            xq_tiles[q] = t

        for q in range(4):
            load_quad(q)

        hhp = ctx.enter_context(tc.tile_pool(name="hhp", bufs=RL))
        osp = ctx.enter_context(tc.tile_pool(name="osp", bufs=3))
        ps1 = ctx.enter_context(tc.tile_pool(name="ps1", bufs=3, space="PSUM"))
        ps2 = ctx.enter_context(tc.tile_pool(name="ps2", bufs=2, space="PSUM"))

        hh_ring = [None] * RL
        hps_ring = [None] * 3
        hps = None
        ops = None

        def eta_op(P):
            # eta(P) = fp16(relu(h) - H); emitted one pair late so it runs
            # on DVE concurrently with the next pair's H-op on Act
            hh_prev = hh_ring[P % RL]
            nc.vector.scalar_tensor_tensor(
                out=hh_prev[64:128, :], in0=hps_ring[P % 3][0:64, :],
                scalar=0.0, in1=hh_prev[0:64, :],
                op0=Alu.max, op1=Alu.subtract,
            )

        NP = NG // 2          # 256 pairs
        PLAG = LAG // 2       # lag in pairs
        for P in range(NP + PLAG):
            if P < NP:
                # ---- stage1 block: 4 same-shape matmuls back to back
                hps = ps1.tile([64, 1024], f32, tag="hps")
                hps_ring[P % 3] = hps
                for pj in (0, 1):
                    g = 2 * P + pj
                    q, j4 = g // 4, g % 4
                    if j4 == 0:
                        load_quad(q + 4)
                    xq = xq_tiles[q]
                    dst = hps[0:64, pj * 512:(pj + 1) * 512]
                    rhs = xq[0:112, j4 * 512:(j4 + 1) * 512]
                    nc.tensor.matmul(out=dst, lhsT=w1a_s[:], rhs=rhs,
                                     start=True, stop=False)
                    nc.tensor.matmul(out=dst, lhsT=w1b_s[:], rhs=rhs,
                                     start=False, stop=True)
                hh = hhp.tile([128, 1024], fp16, tag="hh")
                # H = fp16(relu(h)) on Act; eta lags a pair on DVE
                nc.scalar.activation(out=hh[0:64, :], in_=hps[0:64, :],
                                     func=Relu)
                hh_ring[P % RL] = hh
                if P >= 1:
                    eta_op(P - 1)
                if P == NP - 1:
                    eta_op(P)
            if P >= PLAG:
                # ---- stage2 block: 4 K=128 matmuls back to back
                P2 = P - PLAG
                for pj2 in (0, 1):
                    g2 = 2 * P2 + pj2
                    j4 = g2 % 4
                    if j4 == 0:
                        ops = ps2.tile([128, 512], f32, tag="ops")
                    hh2 = hh_ring[P2 % RL]
                    rhs2 = hh2[0:128, pj2 * 512:(pj2 + 1) * 512]
                    q0 = j4 * 32
                    nc.tensor.matmul(out=ops[q0:q0 + 32, :], lhsT=w2a_s[:],
                                     rhs=rhs2, start=True, stop=False,
                                     tile_position=(0, q0))
                    nc.tensor.matmul(out=ops[q0:q0 + 32, :], lhsT=w2b_s[:],
                                     rhs=rhs2, start=False, stop=True,
                                     tile_position=(0, q0))
                    if j4 == 3:
                        qq = g2 // 4
                        osb = osp.tile([128, 512], f32, tag="osb")
                        if qq % 2 == 0:
                            nc.scalar.copy(out=osb[:], in_=ops[:])
                        else:
                            nc.vector.tensor_copy(out=osb[:], in_=ops[:])
                        nc.sync.dma_start(
                            out=out2[:, qq * 512:(qq + 1) * 512], in_=osb[:]
                        )


# ----------------------------------------------------------------- host prep
def _split16(x):
    hi = x.astype(np.float16)
    lo = (x - hi.astype(np.float32)).astype(np.float16)
    return hi, lo


def prep_weights(inputs):
    w1 = np.zeros((56, 64), np.float32)
    w1[0:28, 0:32] = np.asarray(inputs["ang_w1"], np.float32)
    w1[28:56, 32:64] = np.asarray(inputs["md_w1"], np.float32)
    b1 = np.concatenate([np.asarray(inputs["ang_b1"]),
                         np.asarray(inputs["md_b1"])]).astype(np.float32)
    assert not np.any(b1), "kernel assumes zero hidden bias"
    w2 = np.concatenate([np.asarray(inputs["ang_w2"], np.float32),
                         np.asarray(inputs["md_w2"], np.float32)], 0)
    A1, a1 = _split16(w1)
    A2, a2 = _split16(w2)
    w1a = np.concatenate([A1, A1], 0)                       # [112, 64]
    w1b = np.concatenate([a1, a1], 0)                       # [112, 64]
    w2a = np.concatenate([A2, A2], 0)                       # [128, 32]
    w2b = np.concatenate([a2, a2], 0)                       # [128, 32]
    b2 = (np.asarray(inputs["ang_b2"]) + np.asarray(inputs["md_b2"]))
    return w1a, w1b, w2a, w2b, b2.astype(np.float32)


def prep_x(g, inputs):
    ang = np.asarray(inputs["angle"][g], np.float32).reshape(T, 28)
    dst = np.asarray(inputs["dists"][g], np.float32).reshape(T, 28)
    x = np.concatenate([ang, dst], 1).T                     # [56, T]
    X, xi = _split16(np.ascontiguousarray(x))
    return np.concatenate([X, xi], 0)                       # [112, T]


def edge_bias_host(g, inputs, full):
    """Exact f32 edge-embedding scatter, matching the reference."""
    ef = np.asarray(inputs["edge_feat"][g], np.float32)
    ei = np.asarray(inputs["edge_index"][g]).astype(np.int64)
    mask = np.asarray(inputs["edge_mask"][g]).astype(bool)
    nlig = max(int(inputs["num_ligand_atoms"][g]), 1)

    t0 = ef[:, 0].astype(np.int32)
    t1 = ef[:, 1].astype(np.int32)
    t2 = ef[:, 2].astype(np.int32)
    d = ef[:, 3:4]                                          # [E,1]
    src, tgt = ei[0], ei[1]
    src_l = (src > 0) & (src < nlig)
    tgt_l = (tgt > 0) & (tgt < nlig)

    dw1 = np.asarray(inputs["dist_w1"], np.float32)
    db1 = np.asarray(inputs["dist_b1"], np.float32)
    dw2 = np.asarray(inputs["dist_w2"], np.float32)
    db2 = np.asarray(inputs["dist_b2"], np.float32)
    demb = np.maximum(d @ dw1 + db1, 0.0) @ dw2 + db2       # [E, 32]

    sidx = np.clip(t0 * 4 + t1 * 2 + t2, 0, 19)
    structural = np.asarray(inputs["struct_emb"], np.float32)[sidx]
    pidx = np.clip(t1, 0, 14)
    both_l = src_l & tgt_l
    both_p = (~src_l) & (~tgt_l)
    plip = np.where(
        both_l[:, None], np.asarray(inputs["plip_lig"], np.float32)[pidx],
        np.where(both_p[:, None], np.asarray(inputs["plip_prot"], np.float32)[pidx],
                 np.asarray(inputs["plip_inter"], np.float32)[pidx]))
    emb = np.where((t0 <= 1)[:, None], structural,
                   np.where((t0 == 5)[:, None], plip, 0.0)) + demb
    emb = emb * mask[:, None].astype(np.float32)

    flat = full.reshape(-1)
    cell = ((src + 1) * NP1 + (tgt + 1)).astype(np.int64)   # [E]
    idx = (np.arange(H, dtype=np.int64) * (NP1 * NP1))[None, :] + cell[:, None]
    np.add.at(flat, idx.ravel(), emb.astype(np.float32).ravel())


_IN_SPECS = [
    ("xln", (112, T), "float16"),
    ("w1a", (112, 64), "float16"),
    ("w1b", (112, 64), "float16"),
    ("w2a", (128, 32), "float16"),
    ("w2b", (128, 32), "float16"),
]


def _build_nc():
    from concourse import bacc, mybir

    nc = bacc.Bacc(
        "TRN2",
        target_bir_lowering=False,
        debug=False,
        enable_asserts=False,
        num_devices=8,
    )
    ins = {}
    for name, shape, dt in _IN_SPECS:
        h = nc.dram_tensor(name, list(shape), getattr(mybir.dt, dt),
                           kind="ExternalInput")
        ins[name] = h[:]
    out_h = nc.dram_tensor("out", [128, T // 4], mybir.dt.float32,
                           kind="ExternalOutput")
    build(nc, {"out": out_h[:]}, ins)
    nc.compile()
    return nc


def kernel(_trace=False, **inputs):
    from concourse.bass_utils import run_bass_kernel_spmd

    w1a, w1b, w2a, w2b, b2 = prep_weights(inputs)
    in_maps = []
    for g in range(G):
        in_maps.append(dict(xln=prep_x(g, inputs), w1a=w1a, w1b=w1b,
                            w2a=w2a, w2b=w2b))

    nc = _build_nc()
    res = run_bass_kernel_spmd(nc, in_maps, core_ids=list(range(G)),
                               trace=_trace)
    if _trace:
        print("HW exec time:", res.exec_time_ns, "ns  (mean:",
              res.mean_exec_time_ns, "ns, slowest core:",
              res.max_exec_time_core_id, ")")
        if res.instructions_and_trace:
            print("trace:", res.instructions_and_trace[1])

    attn = np.asarray(inputs["attn_bias"], np.float32)      # [G, 513, 513]
    virt = np.asarray(inputs["virt"], np.float32).reshape(H)
    outs = []
    for g, r in enumerate(res.results):
        # device rows: (g%4)*32 + h; cols: (g//4)*512 + c
        dev = r["out"].reshape(4, 32, NQ, 512).transpose(1, 2, 0, 3)
        dev = dev.reshape(32, T).reshape(32, N, N)          # [H, i, j]
        full = np.empty((H, NP1, NP1), np.float32)
        full[:, 1:, 1:] = dev + b2[:, None, None] + attn[g][None, 1:, 1:]
        full[:, 1:, 0] = attn[g][None, 1:, 0] + virt[:, None]
        full[:, 0, :] = attn[g][None, 0, :] + virt[:, None]
        edge_bias_host(g, inputs, full)
        outs.append(full)
    return np.stack(outs).astype(np.float32)


# revision 8
# speedup vs baseline: 2.9146x; 1.1054x over previous
import sys

sys.path.insert(0, "/opt/trn_rl_repo")

import numpy as np

G, E, N, H = 8, 8192, 512, 32
NP1 = N + 1          # 513
T = N * N            # 262144 tokens per graph
NG = 512             # token groups of 512
NQ = NG // 4         # 128 quads
LAG = 8              # stage1 -> stage2 lag in groups (multiple of 4)
RL = LAG // 2 + 4    # hh ring size in pairs


# ----------------------------------------------------------------- device code
def build(nc, outs, ins):
    from contextlib import ExitStack

    import concourse.tile as tile
    from concourse import mybir

    f32 = mybir.dt.float32
    fp16 = mybir.dt.float16
    Relu = mybir.ActivationFunctionType.Relu
    Alu = mybir.AluOpType

    out2 = outs["out"]               # [128, T//4] f32; row = (g%4)*32+h
    xln = ins["xln"]                 # [112, T] fp16 (X rows 0:56, xi rows 56:112)
    w1a = ins["w1a"]                 # [112, 64] fp16 = [A1; A1]
    w1b = ins["w1b"]                 # [112, 64] fp16 = [a1; a1]
    w2a = ins["w2a"]                 # [128, 32] fp16 = [A2; A2]
    w2b = ins["w2b"]                 # [128, 32] fp16 = [a2; a2]

    with tile.TileContext(nc) as tc, ExitStack() as ctx:
        cst = ctx.enter_context(tc.tile_pool(name="cst", bufs=1))
        w1a_s = cst.tile([112, 64], fp16)
        nc.sync.dma_start(out=w1a_s[:], in_=w1a[:])
        w1b_s = cst.tile([112, 64], fp16)
        nc.sync.dma_start(out=w1b_s[:], in_=w1b[:])
        w2a_s = cst.tile([128, 32], fp16)
        nc.sync.dma_start(out=w2a_s[:], in_=w2a[:])
        w2b_s = cst.tile([128, 32], fp16)
        nc.sync.dma_start(out=w2b_s[:], in_=w2b[:])

        # x input staged per quad (4 groups) via SWDGE striping
        xin = ctx.enter_context(tc.tile_pool(name="xin", bufs=5))
        xq_tiles = {}

        def load_quad(q):
            if q >= NQ or q in xq_tiles:
                return
            t = xin.tile([112, 2048], fp16, tag="xq")
            nc.gpsimd.dma_start(out=t[:], in_=xln[:, q * 2048:(q + 1) * 2048])
            xq_tiles[q] = t

        for q in range(4):
            load_quad(q)

        hhp = ctx.enter_context(tc.tile_pool(name="hhp", bufs=RL))
        osp = ctx.enter_context(tc.tile_pool(name="osp", bufs=3))
        ps1 = ctx.enter_context(tc.tile_pool(name="ps1", bufs=3, space="PSUM"))
        ps2 = ctx.enter_context(tc.tile_pool(name="ps2", bufs=2, space="PSUM"))

        hh_ring = [None] * RL
        hps_ring = [None] * 3
        hps = None
        ops = None

        def eta_op(P):
            # eta(P) = fp16(relu(h) - H); emitted one pair late so it runs
            # on DVE concurrently with the next pair's H-op on Act
            hh_prev = hh_ring[P % RL]
            nc.vector.scalar_tensor_tensor(
                out=hh_prev[64:128, :], in0=hps_ring[P % 3][0:64, :],
                scalar=0.0, in1=hh_prev[0:64, :],
                op0=Alu.max, op1=Alu.subtract,
            )

        NB = NG // 4          # 128 blocks of 4 groups (2 pairs)
        BLAG = LAG // 4       # lag in blocks

        def stage1_pair(P):
            hps = ps1.tile([64, 1024], f32, tag="hps")
            hps_ring[P % 3] = hps
            for pj in (0, 1):
                g = 2 * P + pj
                q, j4 = g // 4, g % 4
                if j4 == 0:
                    load_quad(q + 4)
                xq = xq_tiles[q]
                dst = hps[0:64, pj * 512:(pj + 1) * 512]
                rhs = xq[0:112, j4 * 512:(j4 + 1) * 512]
                nc.tensor.matmul(out=dst, lhsT=w1a_s[:], rhs=rhs,
                                 start=True, stop=False)
                nc.tensor.matmul(out=dst, lhsT=w1b_s[:], rhs=rhs,
                                 start=False, stop=True)

        def h_op(P):
            hh = hhp.tile([128, 1024], fp16, tag="hh")
            nc.scalar.activation(out=hh[0:64, :],
                                 in_=hps_ring[P % 3][0:64, :], func=Relu)
            hh_ring[P % RL] = hh

        for B in range(NB + BLAG):
            if B < NB:
                # ---- 8 stage1 matmuls back to back (2 pairs)
                stage1_pair(2 * B)
                stage1_pair(2 * B + 1)
                # H on Act; eta lags a pair, runs on DVE in parallel
                h_op(2 * B)
                h_op(2 * B + 1)
                if B >= 1:
                    eta_op(2 * B - 1)
                eta_op(2 * B)
                if B == NB - 1:
                    eta_op(2 * B + 1)
            if B >= BLAG:
                # ---- 8 stage2 K=128 matmuls back to back (one quad)
                B2 = B - BLAG
                ops = ps2.tile([128, 512], f32, tag="ops")
                for j4 in range(4):
                    g2 = 4 * B2 + j4
                    P2, pj2 = g2 // 2, g2 % 2
                    hh2 = hh_ring[P2 % RL]
                    rhs2 = hh2[0:128, pj2 * 512:(pj2 + 1) * 512]
                    q0 = j4 * 32
                    nc.tensor.matmul(out=ops[q0:q0 + 32, :], lhsT=w2a_s[:],
                                     rhs=rhs2, start=True, stop=False,
                                     tile_position=(0, q0))
                    nc.tensor.matmul(out=ops[q0:q0 + 32, :], lhsT=w2b_s[:],
                                     rhs=rhs2, start=False, stop=True,
                                     tile_position=(0, q0))
                osb = osp.tile([128, 512], f32, tag="osb")
                if B2 % 3 == 2:
                    nc.vector.tensor_copy(out=osb[:], in_=ops[:])
                else:
                    nc.scalar.copy(out=osb[:], in_=ops[:])
                nc.sync.dma_start(out=out2[:, B2 * 512:(B2 + 1) * 512],
                                  in_=osb[:])


# ----------------------------------------------------------------- host prep
def _split16(x):
    hi = x.astype(np.float16)
    lo = (x - hi.astype(np.float32)).astype(np.float16)
    return hi, lo


def prep_weights(inputs):
    w1 = np.zeros((56, 64), np.float32)
    w1[0:28, 0:32] = np.asarray(inputs["ang_w1"], np.float32)
    w1[28:56, 32:64] = np.asarray(inputs["md_w1"], np.float32)
    b1 = np.concatenate([np.asarray(inputs["ang_b1"]),
                         np.asarray(inputs["md_b1"])]).astype(np.float32)
    assert not np.any(b1), "kernel assumes zero hidden bias"
    w2 = np.concatenate([np.asarray(inputs["ang_w2"], np.float32),
                         np.asarray(inputs["md_w2"], np.float32)], 0)
    A1, a1 = _split16(w1)
    A2, a2 = _split16(w2)
    w1a = np.concatenate([A1, A1], 0)                       # [112, 64]
    w1b = np.concatenate([a1, a1], 0)                       # [112, 64]
    w2a = np.concatenate([A2, A2], 0)                       # [128, 32]
    w2b = np.concatenate([a2, a2], 0)                       # [128, 32]
    b2 = (np.asarray(inputs["ang_b2"]) + np.asarray(inputs["md_b2"]))
    return w1a, w1b, w2a, w2b, b2.astype(np.float32)


def prep_x(g, inputs):
    ang = np.asarray(inputs["angle"][g], np.float32).reshape(T, 28)
    dst = np.asarray(inputs["dists"][g], np.float32).reshape(T, 28)
    x = np.concatenate([ang, dst], 1).T                     # [56, T]
    X, xi = _split16(np.ascontiguousarray(x))
    return np.concatenate([X, xi], 0)                       # [112, T]


def edge_bias_host(g, inputs, full):
    """Exact f32 edge-embedding scatter, matching the reference."""
    ef = np.asarray(inputs["edge_feat"][g], np.float32)
    ei = np.asarray(inputs["edge_index"][g]).astype(np.int64)
    mask = np.asarray(inputs["edge_mask"][g]).astype(bool)
    nlig = max(int(inputs["num_ligand_atoms"][g]), 1)

    t0 = ef[:, 0].astype(np.int32)
    t1 = ef[:, 1].astype(np.int32)
    t2 = ef[:, 2].astype(np.int32)
    d = ef[:, 3:4]                                          # [E,1]
    src, tgt = ei[0], ei[1]
    src_l = (src > 0) & (src < nlig)
    tgt_l = (tgt > 0) & (tgt < nlig)

    dw1 = np.asarray(inputs["dist_w1"], np.float32)
    db1 = np.asarray(inputs["dist_b1"], np.float32)
    dw2 = np.asarray(inputs["dist_w2"], np.float32)
    db2 = np.asarray(inputs["dist_b2"], np.float32)
    demb = np.maximum(d @ dw1 + db1, 0.0) @ dw2 + db2       # [E, 32]

    sidx = np.clip(t0 * 4 + t1 * 2 + t2, 0, 19)
    structural = np.asarray(inputs["struct_emb"], np.float32)[sidx]
    pidx = np.clip(t1, 0, 14)
    both_l = src_l & tgt_l
    both_p = (~src_l) & (~tgt_l)
    plip = np.where(
        both_l[:, None], np.asarray(inputs["plip_lig"], np.float32)[pidx],
        np.where(both_p[:, None], np.asarray(inputs["plip_prot"], np.float32)[pidx],
                 np.asarray(inputs["plip_inter"], np.float32)[pidx]))
    emb = np.where((t0 <= 1)[:, None], structural,
                   np.where((t0 == 5)[:, None], plip, 0.0)) + demb
    emb = emb * mask[:, None].astype(np.float32)

    flat = full.reshape(-1)
    cell = ((src + 1) * NP1 + (tgt + 1)).astype(np.int64)   # [E]
    idx = (np.arange(H, dtype=np.int64) * (NP1 * NP1))[None, :] + cell[:, None]
    np.add.at(flat, idx.ravel(), emb.astype(np.float32).ravel())


_IN_SPECS = [
    ("xln", (112, T), "float16"),
    ("w1a", (112, 64), "float16"),
    ("w1b", (112, 64), "float16"),
    ("w2a", (128, 32), "float16"),
    ("w2b", (128, 32), "float16"),
]


def _build_nc():
    from concourse import bacc, mybir

    nc = bacc.Bacc(
        "TRN2",
        target_bir_lowering=False,
        debug=False,
        enable_asserts=False,
        num_devices=8,
    )
    ins = {}
    for name, shape, dt in _IN_SPECS:
        h = nc.dram_tensor(name, list(shape), getattr(mybir.dt, dt),
                           kind="ExternalInput")
        ins[name] = h[:]
    out_h = nc.dram_tensor("out", [128, T // 4], mybir.dt.float32,
                           kind="ExternalOutput")
    build(nc, {"out": out_h[:]}, ins)
    nc.compile()
    return nc


def kernel(_trace=False, **inputs):
    from concourse.bass_utils import run_bass_kernel_spmd

    w1a, w1b, w2a, w2b, b2 = prep_weights(inputs)
    in_maps = []
    for g in range(G):
        in_maps.append(dict(xln=prep_x(g, inputs), w1a=w1a, w1b=w1b,
                            w2a=w2a, w2b=w2b))

    nc = _build_nc()
    res = run_bass_kernel_spmd(nc, in_maps, core_ids=list(range(G)),
                               trace=_trace)
    if _trace:
        print("HW exec time:", res.exec_time_ns, "ns  (mean:",
              res.mean_exec_time_ns, "ns, slowest core:",
              res.max_exec_time_core_id, ")")
        if res.instructions_and_trace:
            print("trace:", res.instructions_and_trace[1])

    attn = np.asarray(inputs["attn_bias"], np.float32)      # [G, 513, 513]
    virt = np.asarray(inputs["virt"], np.float32).reshape(H)
    outs = []
    for g, r in enumerate(res.results):
        # device rows: (g%4)*32 + h; cols: (g//4)*512 + c
        dev = r["out"].reshape(4, 32, NQ, 512).transpose(1, 2, 0, 3)
        dev = dev.reshape(32, T).reshape(32, N, N)          # [H, i, j]
        full = np.empty((H, NP1, NP1), np.float32)
        full[:, 1:, 1:] = dev + b2[:, None, None] + attn[g][None, 1:, 1:]
        full[:, 1:, 0] = attn[g][None, 1:, 0] + virt[:, None]
        full[:, 0, :] = attn[g][None, 0, :] + virt[:, None]
        edge_bias_host(g, inputs, full)
        outs.append(full)
    return np.stack(outs).astype(np.float32)


# revision 18
# speedup vs baseline: 3.1203x; 1.0706x over previous
import sys

sys.path.insert(0, "/opt/trn_rl_repo")

import numpy as np

G, E, N, H = 8, 8192, 512, 32
NP1 = N + 1          # 513
T = N * N            # 262144 tokens per graph
NG = 512             # token groups of 512
NQ = NG // 4         # 128 quads
LAG = 8              # stage1 -> stage2 lag in groups (multiple of 4)
RL = LAG // 2 + 4    # hh ring size in pairs


# ----------------------------------------------------------------- device code
def build(nc, outs, ins):
    from contextlib import ExitStack

    import concourse.tile as tile
    from concourse import mybir

    f32 = mybir.dt.float32
    fp16 = mybir.dt.float16
    Relu = mybir.ActivationFunctionType.Relu
    Alu = mybir.AluOpType

    out2 = outs["out"]               # [128, T//4] f32; row = (g%4)*32+h
    xln = ins["xln"]                 # [112, T] fp16 (X rows 0:56, xi rows 56:112)
    w1a = ins["w1a"]                 # [112, 64] fp16 = [A1; A1]
    w1b = ins["w1b"]                 # [112, 64] fp16 = [a1; a1]
    w2a = ins["w2a"]                 # [128, 64] fp16 = [[A2;0] | [0;A2]]
    w2b = ins["w2b"]                 # [128, 64] fp16 = [[a2;0] | [0;a2]]

    with tile.TileContext(nc) as tc, ExitStack() as ctx:
        cst = ctx.enter_context(tc.tile_pool(name="cst", bufs=1))
        w1a_s = cst.tile([112, 64], fp16)
        nc.sync.dma_start(out=w1a_s[:], in_=w1a[:])
        w1b_s = cst.tile([112, 64], fp16)
        nc.sync.dma_start(out=w1b_s[:], in_=w1b[:])
        w2a_s = cst.tile([128, 64], fp16)
        nc.sync.dma_start(out=w2a_s[:], in_=w2a[:])
        w2b_s = cst.tile([128, 64], fp16)
        nc.sync.dma_start(out=w2b_s[:], in_=w2b[:])

        # x input staged per quad (4 groups) via SWDGE striping
        xin = ctx.enter_context(tc.tile_pool(name="xin", bufs=5))
        xq_tiles = {}

        def load_quad(q):
            if q >= NQ or q in xq_tiles:
                return
            t = xin.tile([112, 2048], fp16, tag="xq")
            nc.gpsimd.dma_start(out=t[:], in_=xln[:, q * 2048:(q + 1) * 2048])
            xq_tiles[q] = t

        for q in range(4):
            load_quad(q)

        hhp = ctx.enter_context(tc.tile_pool(name="hhp", bufs=RL))
        etp = ctx.enter_context(tc.tile_pool(name="etp", bufs=RL))
        osp = ctx.enter_context(tc.tile_pool(name="osp", bufs=3))
        ps1 = ctx.enter_context(tc.tile_pool(name="ps1", bufs=4, space="PSUM"))
        ps2 = ctx.enter_context(tc.tile_pool(name="ps2", bufs=3, space="PSUM"))

        ht_ring = [None] * RL            # H pair tiles [128, 512]
        et_ring = [None] * RL            # eta pair tiles [128, 512]
        hps_ring = [None] * 4

        def eta_op(P):
            # eta(P) = fp16(relu(h) - H); lags a pair so it runs on DVE
            # concurrently with the next pair's H-op on Act
            nc.vector.scalar_tensor_tensor(
                out=et_ring[P % RL][:], in0=hps_ring[P % 4][:],
                scalar=0.0, in1=ht_ring[P % RL][:],
                op0=Alu.max, op1=Alu.subtract,
            )

        NB = NG // 4          # 128 blocks of 4 groups (2 pairs)
        BLAG = LAG // 4       # lag in blocks

        def stage1_pair(P):
            # pair-packed psum [128, 512]: rows 0:64 = h(2P), 64:128 = h(2P+1)
            # 8 matmuls on 4 disjoint 32-wide PE column tiles -> concurrent
            hps = ps1.tile([128, 512], f32, tag="hps")
            hps_ring[P % 4] = hps
            rhs = []
            for pj in (0, 1):
                g = 2 * P + pj
                q, j4 = g // 4, g % 4
                if j4 == 0:
                    load_quad(q + 4)
                xq = xq_tiles[q]
                rhs.append(xq[0:112, j4 * 512:(j4 + 1) * 512])
            for c in range(4):
                q0 = 32 * c
                nc.tensor.matmul(out=hps[q0:q0 + 32, :],
                                 lhsT=w1a_s[:, (c % 2) * 32:(c % 2) * 32 + 32],
                                 rhs=rhs[c // 2],
                                 start=True, stop=False,
                                 tile_position=(0, q0),
                                 skip_group_check=True)
            for c in range(4):
                q0 = 32 * c
                nc.tensor.matmul(out=hps[q0:q0 + 32, :],
                                 lhsT=w1b_s[:, (c % 2) * 32:(c % 2) * 32 + 32],
                                 rhs=rhs[c // 2],
                                 start=False, stop=True,
                                 tile_position=(0, q0),
                                 skip_group_check=True)

        def h_op(P):
            ht = hhp.tile([128, 512], fp16, tag="ht")
            et = etp.tile([128, 512], fp16, tag="et")
            nc.scalar.activation(out=ht[:], in_=hps_ring[P % 4][:], func=Relu)
            ht_ring[P % RL] = ht
            et_ring[P % RL] = et

        for B in range(NB + BLAG):
            if B < NB:
                # ---- stage1: 2 pairs, col-tiled concurrent matmul pairs
                stage1_pair(2 * B)
                stage1_pair(2 * B + 1)
                h_op(2 * B)
                h_op(2 * B + 1)
                if B >= 1:
                    eta_op(2 * B - 1)
                eta_op(2 * B)
                if B == NB - 1:
                    eta_op(2 * B + 1)
            if B >= BLAG:
                # ---- stage2: 16 K=64 matmuls on 4 disjoint PE tiles
                B2 = B - BLAG
                ops = ps2.tile([128, 512], f32, tag="ops")
                for j4 in range(4):
                    g2 = 4 * B2 + j4
                    P2, pj2 = g2 // 2, g2 % 2
                    w0 = 32 * pj2     # weight col-block selects even/odd rows
                    ht2 = ht_ring[P2 % RL]
                    et2 = et_ring[P2 % RL]
                    q0 = j4 * 32
                    tp = (0, q0)
                    dst = ops[q0:q0 + 32, :]
                    nc.tensor.matmul(out=dst, lhsT=w2a_s[:, w0:w0 + 32],
                                     rhs=ht2[:], start=True, stop=False,
                                     tile_position=tp)
                    nc.tensor.matmul(out=dst, lhsT=w2a_s[:, w0:w0 + 32],
                                     rhs=et2[:], start=False, stop=False,
                                     tile_position=tp)
                    nc.tensor.matmul(out=dst, lhsT=w2b_s[:, w0:w0 + 32],
                                     rhs=ht2[:], start=False, stop=False,
                                     tile_position=tp)
                    nc.tensor.matmul(out=dst, lhsT=w2b_s[:, w0:w0 + 32],
                                     rhs=et2[:], start=False, stop=True,
                                     tile_position=tp)
                osb = osp.tile([128, 512], f32, tag="osb")
                if B2 % 3 == 2:
                    nc.vector.tensor_copy(out=osb[:], in_=ops[:])
                else:
                    nc.scalar.copy(out=osb[:], in_=ops[:])
                nc.sync.dma_start(out=out2[:, B2 * 512:(B2 + 1) * 512],
                                  in_=osb[:])


# ----------------------------------------------------------------- host prep
def _split16(x):
    hi = x.astype(np.float16)
    lo = (x - hi.astype(np.float32)).astype(np.float16)
    return hi, lo


def prep_weights(inputs):
    w1 = np.zeros((56, 64), np.float32)
    w1[0:28, 0:32] = np.asarray(inputs["ang_w1"], np.float32)
    w1[28:56, 32:64] = np.asarray(inputs["md_w1"], np.float32)
    b1 = np.concatenate([np.asarray(inputs["ang_b1"]),
                         np.asarray(inputs["md_b1"])]).astype(np.float32)
    assert not np.any(b1), "kernel assumes zero hidden bias"
    w2 = np.concatenate([np.asarray(inputs["ang_w2"], np.float32),
                         np.asarray(inputs["md_w2"], np.float32)], 0)
    A1, a1 = _split16(w1)
    A2, a2 = _split16(w2)
    w1a = np.concatenate([A1, A1], 0)                       # [112, 64]
    w1b = np.concatenate([a1, a1], 0)                       # [112, 64]
    z = np.zeros((64, 32), np.float16)
    w2a = np.block([[A2, z], [z, A2]])                      # [128, 64]
    w2b = np.block([[a2, z], [z, a2]])                      # [128, 64]
    b2 = (np.asarray(inputs["ang_b2"]) + np.asarray(inputs["md_b2"]))
    return w1a, w1b, w2a, w2b, b2.astype(np.float32)


def prep_x(g, inputs):
    ang = np.asarray(inputs["angle"][g], np.float32).reshape(T, 28)
    dst = np.asarray(inputs["dists"][g], np.float32).reshape(T, 28)
    x = np.concatenate([ang, dst], 1).T                     # [56, T]
    X, xi = _split16(np.ascontiguousarray(x))
    return np.concatenate([X, xi], 0)                       # [112, T]


def edge_bias_host(g, inputs, full):
    """Exact f32 edge-embedding scatter, matching the reference."""
    ef = np.asarray(inputs["edge_feat"][g], np.float32)
    ei = np.asarray(inputs["edge_index"][g]).astype(np.int64)
    mask = np.asarray(inputs["edge_mask"][g]).astype(bool)
    nlig = max(int(inputs["num_ligand_atoms"][g]), 1)

    t0 = ef[:, 0].astype(np.int32)
    t1 = ef[:, 1].astype(np.int32)
    t2 = ef[:, 2].astype(np.int32)
    d = ef[:, 3:4]                                          # [E,1]
    src, tgt = ei[0], ei[1]
    src_l = (src > 0) & (src < nlig)
    tgt_l = (tgt > 0) & (tgt < nlig)

    dw1 = np.asarray(inputs["dist_w1"], np.float32)
    db1 = np.asarray(inputs["dist_b1"], np.float32)
    dw2 = np.asarray(inputs["dist_w2"], np.float32)
    db2 = np.asarray(inputs["dist_b2"], np.float32)
    demb = np.maximum(d @ dw1 + db1, 0.0) @ dw2 + db2       # [E, 32]

    sidx = np.clip(t0 * 4 + t1 * 2 + t2, 0, 19)
    structural = np.asarray(inputs["struct_emb"], np.float32)[sidx]
    pidx = np.clip(t1, 0, 14)
    both_l = src_l & tgt_l
    both_p = (~src_l) & (~tgt_l)
    plip = np.where(
        both_l[:, None], np.asarray(inputs["plip_lig"], np.float32)[pidx],
        np.where(both_p[:, None], np.asarray(inputs["plip_prot"], np.float32)[pidx],
                 np.asarray(inputs["plip_inter"], np.float32)[pidx]))
    emb = np.where((t0 <= 1)[:, None], structural,
                   np.where((t0 == 5)[:, None], plip, 0.0)) + demb
    emb = emb * mask[:, None].astype(np.float32)

    flat = full.reshape(-1)
    cell = ((src + 1) * NP1 + (tgt + 1)).astype(np.int64)   # [E]
    idx = (np.arange(H, dtype=np.int64) * (NP1 * NP1))[None, :] + cell[:, None]
    np.add.at(flat, idx.ravel(), emb.astype(np.float32).ravel())


_IN_SPECS = [
    ("xln", (112, T), "float16"),
    ("w1a", (112, 64), "float16"),
    ("w1b", (112, 64), "float16"),
    ("w2a", (128, 64), "float16"),
    ("w2b", (128, 64), "float16"),
]


def _build_nc():
    from concourse import bacc, mybir

    nc = bacc.Bacc(
        "TRN2",
        target_bir_lowering=False,
        debug=False,
        enable_asserts=False,
        num_devices=8,
    )
    ins = {}
    for name, shape, dt in _IN_SPECS:
        h = nc.dram_tensor(name, list(shape), getattr(mybir.dt, dt),
                           kind="ExternalInput")
        ins[name] = h[:]
    out_h = nc.dram_tensor("out", [128, T // 4], mybir.dt.float32,
                           kind="ExternalOutput")
    build(nc, {"out": out_h[:]}, ins)
    nc.compile()
    return nc


def kernel(_trace=False, **inputs):
    from concourse.bass_utils import run_bass_kernel_spmd

    w1a, w1b, w2a, w2b, b2 = prep_weights(inputs)
    in_maps = []
    for g in range(G):
        in_maps.append(dict(xln=prep_x(g, inputs), w1a=w1a, w1b=w1b,
                            w2a=w2a, w2b=w2b))

    nc = _build_nc()
    res = run_bass_kernel_spmd(nc, in_maps, core_ids=list(range(G)),
                               trace=_trace)
    if _trace:
        print("HW exec time:", res.exec_time_ns, "ns  (mean:",
              res.mean_exec_time_ns, "ns, slowest core:",
              res.max_exec_time_core_id, ")")
        if res.instructions_and_trace:
            print("trace:", res.instructions_and_trace[1])

    attn = np.asarray(inputs["attn_bias"], np.float32)      # [G, 513, 513]
    virt = np.asarray(inputs["virt"], np.float32).reshape(H)
    outs = []
    for g, r in enumerate(res.results):
        # device rows: (g%4)*32 + h; cols: (g//4)*512 + c
        dev = r["out"].reshape(4, 32, NQ, 512).transpose(1, 2, 0, 3)
        dev = dev.reshape(32, T).reshape(32, N, N)          # [H, i, j]
        full = np.empty((H, NP1, NP1), np.float32)
        full[:, 1:, 1:] = dev + b2[:, None, None] + attn[g][None, 1:, 1:]
        full[:, 1:, 0] = attn[g][None, 1:, 0] + virt[:, None]
        full[:, 0, :] = attn[g][None, 0, :] + virt[:, None]
        edge_bias_host(g, inputs, full)
        outs.append(full)
    return np.stack(outs).astype(np.float32)


# revision 21
# speedup vs baseline: 3.3797x; 1.0831x over previous
import sys

sys.path.insert(0, "/opt/trn_rl_repo")

import numpy as np

G, E, N, H = 8, 8192, 512, 32
NP1 = N + 1          # 513
T = N * N            # 262144 tokens per graph
NG = 512             # token groups of 512
NQ = NG // 4         # 128 quads
LAG = 8              # stage1 -> stage2 lag in groups (multiple of 4)
RL = LAG // 2 + 4    # hh ring size in pairs


# ----------------------------------------------------------------- device code
def build(nc, outs, ins):
    from contextlib import ExitStack

    import concourse.tile as tile
    from concourse import mybir

    f32 = mybir.dt.float32
    fp16 = mybir.dt.float16
    Relu = mybir.ActivationFunctionType.Relu
    Alu = mybir.AluOpType

    out2 = outs["out"]               # [128, T//4] f32; row = (g%4)*32+h
    xln = ins["xln"]                 # [112, T] fp16 (X rows 0:56, xi rows 56:112)
    w1a = ins["w1a"]                 # [112, 64] fp16 = [A1; A1]
    w1b = ins["w1b"]                 # [112, 64] fp16 = [a1; a1]
    w2a = ins["w2a"]                 # [128, 64] fp16 = [[A2;0] | [0;A2]]
    w2b = ins["w2b"]                 # [128, 64] fp16 = [[a2;0] | [0;a2]]

    with tile.TileContext(nc) as tc, ExitStack() as ctx:
        cst = ctx.enter_context(tc.tile_pool(name="cst", bufs=1))
        w1a_s = cst.tile([112, 64], fp16)
        nc.sync.dma_start(out=w1a_s[:], in_=w1a[:])
        w1b_s = cst.tile([112, 64], fp16)
        nc.sync.dma_start(out=w1b_s[:], in_=w1b[:])
        w2a_s = cst.tile([128, 64], fp16)
        nc.sync.dma_start(out=w2a_s[:], in_=w2a[:])
        w2b_s = cst.tile([128, 64], fp16)
        nc.sync.dma_start(out=w2b_s[:], in_=w2b[:])

        # x input staged per oct (8 groups) via SWDGE striping
        xin = ctx.enter_context(tc.tile_pool(name="xin", bufs=4))
        xo_tiles = {}

        def load_oct(o):
            if o >= NQ // 2 or o in xo_tiles:
                return
            t = xin.tile([112, 4096], fp16, tag="xo")
            nc.gpsimd.dma_start(out=t[:], in_=xln[:, o * 4096:(o + 1) * 4096])
            xo_tiles[o] = t

        for o in range(2):
            load_oct(o)

        hhp = ctx.enter_context(tc.tile_pool(name="hhp", bufs=RL))
        etp = ctx.enter_context(tc.tile_pool(name="etp", bufs=RL))
        osp = ctx.enter_context(tc.tile_pool(name="osp", bufs=3))
        ps1 = ctx.enter_context(tc.tile_pool(name="ps1", bufs=4, space="PSUM"))
        ps2 = ctx.enter_context(tc.tile_pool(name="ps2", bufs=3, space="PSUM"))

        ht_ring = [None] * RL            # H pair tiles [128, 512]
        et_ring = [None] * RL            # eta pair tiles [128, 512]
        hps_ring = [None] * 4

        def eta_op(P):
            # eta(P) = fp16(relu(h) - H); lags a pair so it runs on DVE
            # concurrently with the next pair's H-op on Act
            nc.vector.scalar_tensor_tensor(
                out=et_ring[P % RL][:], in0=hps_ring[P % 4][:],
                scalar=0.0, in1=ht_ring[P % RL][:],
                op0=Alu.max, op1=Alu.subtract,
            )

        NB = NG // 4          # 128 blocks of 4 groups (2 pairs)
        BLAG = LAG // 4       # lag in blocks

        def stage1_pair(P):
            # pair-packed psum [128, 512]: rows 0:64 = h(2P), 64:128 = h(2P+1)
            # 8 matmuls on 4 disjoint 32-wide PE column tiles -> concurrent
            hps = ps1.tile([128, 512], f32, tag="hps")
            hps_ring[P % 4] = hps
            rhs = []
            for pj in (0, 1):
                g = 2 * P + pj
                o, j8 = g // 8, g % 8
                if j8 == 0:
                    load_oct(o + 2)
                xo = xo_tiles[o]
                rhs.append(xo[0:112, j8 * 512:(j8 + 1) * 512])
            for c in range(4):
                q0 = 32 * c
                nc.tensor.matmul(out=hps[q0:q0 + 32, :],
                                 lhsT=w1a_s[:, (c % 2) * 32:(c % 2) * 32 + 32],
                                 rhs=rhs[c // 2],
                                 start=True, stop=False,
                                 tile_position=(0, q0),
                                 skip_group_check=True)
            for c in range(4):
                q0 = 32 * c
                nc.tensor.matmul(out=hps[q0:q0 + 32, :],
                                 lhsT=w1b_s[:, (c % 2) * 32:(c % 2) * 32 + 32],
                                 rhs=rhs[c // 2],
                                 start=False, stop=True,
                                 tile_position=(0, q0),
                                 skip_group_check=True)

        def h_op(P):
            ht = hhp.tile([128, 512], fp16, tag="ht")
            et = etp.tile([128, 512], fp16, tag="et")
            nc.scalar.activation(out=ht[:], in_=hps_ring[P % 4][:], func=Relu)
            ht_ring[P % RL] = ht
            et_ring[P % RL] = et

        for B in range(NB + BLAG):
            if B < NB:
                # ---- stage1: 2 pairs, col-tiled concurrent matmul pairs
                stage1_pair(2 * B)
                stage1_pair(2 * B + 1)
                h_op(2 * B)
                h_op(2 * B + 1)
                if B >= 1:
                    eta_op(2 * B - 1)
                eta_op(2 * B)
                if B == NB - 1:
                    eta_op(2 * B + 1)
            if B >= BLAG:
                # ---- stage2: 16 K=64 matmuls on 4 disjoint PE tiles
                B2 = B - BLAG
                ops = ps2.tile([128, 512], f32, tag="ops")
                for j4 in range(4):
                    g2 = 4 * B2 + j4
                    P2, pj2 = g2 // 2, g2 % 2
                    w0 = 32 * pj2     # weight col-block selects even/odd rows
                    ht2 = ht_ring[P2 % RL]
                    et2 = et_ring[P2 % RL]
                    q0 = j4 * 32
                    tp = (0, q0)
                    dst = ops[q0:q0 + 32, :]
                    nc.tensor.matmul(out=dst, lhsT=w2a_s[:, w0:w0 + 32],
                                     rhs=ht2[:], start=True, stop=False,
                                     tile_position=tp)
                    nc.tensor.matmul(out=dst, lhsT=w2a_s[:, w0:w0 + 32],
                                     rhs=et2[:], start=False, stop=False,
                                     tile_position=tp)
                    nc.tensor.matmul(out=dst, lhsT=w2b_s[:, w0:w0 + 32],
                                     rhs=ht2[:], start=False, stop=False,
                                     tile_position=tp)
                    nc.tensor.matmul(out=dst, lhsT=w2b_s[:, w0:w0 + 32],
                                     rhs=et2[:], start=False, stop=True,
                                     tile_position=tp)
                # stage output in [128, 1024] tiles; one DMA per two blocks
                if B2 % 2 == 0:
                    osb = osp.tile([128, 1024], f32, tag="osb")
                half = osb[:, (B2 % 2) * 512:(B2 % 2) * 512 + 512]
                if B2 % 3 == 2:
                    nc.vector.tensor_copy(out=half, in_=ops[:])
                else:
                    nc.scalar.copy(out=half, in_=ops[:])
                if B2 % 2 == 1:
                    nc.sync.dma_start(
                        out=out2[:, (B2 - 1) * 512:(B2 + 1) * 512], in_=osb[:]
                    )


# ----------------------------------------------------------------- host prep
def _split16(x):
    hi = x.astype(np.float16)
    lo = (x - hi.astype(np.float32)).astype(np.float16)
    return hi, lo


def prep_weights(inputs):
    w1 = np.zeros((56, 64), np.float32)
    w1[0:28, 0:32] = np.asarray(inputs["ang_w1"], np.float32)
    w1[28:56, 32:64] = np.asarray(inputs["md_w1"], np.float32)
    b1 = np.concatenate([np.asarray(inputs["ang_b1"]),
                         np.asarray(inputs["md_b1"])]).astype(np.float32)
    assert not np.any(b1), "kernel assumes zero hidden bias"
    w2 = np.concatenate([np.asarray(inputs["ang_w2"], np.float32),
                         np.asarray(inputs["md_w2"], np.float32)], 0)
    A1, a1 = _split16(w1)
    A2, a2 = _split16(w2)
    w1a = np.concatenate([A1, A1], 0)                       # [112, 64]
    w1b = np.concatenate([a1, a1], 0)                       # [112, 64]
    z = np.zeros((64, 32), np.float16)
    w2a = np.block([[A2, z], [z, A2]])                      # [128, 64]
    w2b = np.block([[a2, z], [z, a2]])                      # [128, 64]
    b2 = (np.asarray(inputs["ang_b2"]) + np.asarray(inputs["md_b2"]))
    return w1a, w1b, w2a, w2b, b2.astype(np.float32)


def prep_x(g, inputs):
    ang = np.asarray(inputs["angle"][g], np.float32).reshape(T, 28)
    dst = np.asarray(inputs["dists"][g], np.float32).reshape(T, 28)
    x = np.concatenate([ang, dst], 1).T                     # [56, T]
    X, xi = _split16(np.ascontiguousarray(x))
    return np.concatenate([X, xi], 0)                       # [112, T]


def edge_bias_host(g, inputs, full):
    """Exact f32 edge-embedding scatter, matching the reference."""
    ef = np.asarray(inputs["edge_feat"][g], np.float32)
    ei = np.asarray(inputs["edge_index"][g]).astype(np.int64)
    mask = np.asarray(inputs["edge_mask"][g]).astype(bool)
    nlig = max(int(inputs["num_ligand_atoms"][g]), 1)

    t0 = ef[:, 0].astype(np.int32)
    t1 = ef[:, 1].astype(np.int32)
    t2 = ef[:, 2].astype(np.int32)
    d = ef[:, 3:4]                                          # [E,1]
    src, tgt = ei[0], ei[1]
    src_l = (src > 0) & (src < nlig)
    tgt_l = (tgt > 0) & (tgt < nlig)

    dw1 = np.asarray(inputs["dist_w1"], np.float32)
    db1 = np.asarray(inputs["dist_b1"], np.float32)
    dw2 = np.asarray(inputs["dist_w2"], np.float32)
    db2 = np.asarray(inputs["dist_b2"], np.float32)
    demb = np.maximum(d @ dw1 + db1, 0.0) @ dw2 + db2       # [E, 32]

    sidx = np.clip(t0 * 4 + t1 * 2 + t2, 0, 19)
    structural = np.asarray(inputs["struct_emb"], np.float32)[sidx]
    pidx = np.clip(t1, 0, 14)
    both_l = src_l & tgt_l
    both_p = (~src_l) & (~tgt_l)
    plip = np.where(
        both_l[:, None], np.asarray(inputs["plip_lig"], np.float32)[pidx],
        np.where(both_p[:, None], np.asarray(inputs["plip_prot"], np.float32)[pidx],
                 np.asarray(inputs["plip_inter"], np.float32)[pidx]))
    emb = np.where((t0 <= 1)[:, None], structural,
                   np.where((t0 == 5)[:, None], plip, 0.0)) + demb
    emb = emb * mask[:, None].astype(np.float32)

    flat = full.reshape(-1)
    cell = ((src + 1) * NP1 + (tgt + 1)).astype(np.int64)   # [E]
    idx = (np.arange(H, dtype=np.int64) * (NP1 * NP1))[None, :] + cell[:, None]
    np.add.at(flat, idx.ravel(), emb.astype(np.float32).ravel())


_IN_SPECS = [
    ("xln", (112, T), "float16"),
    ("w1a", (112, 64), "float16"),
    ("w1b", (112, 64), "float16"),
    ("w2a", (128, 64), "float16"),
    ("w2b", (128, 64), "float16"),
]


def _build_nc():
    from concourse import bacc, mybir

    nc = bacc.Bacc(
        "TRN2",
        target_bir_lowering=False,
        debug=False,
        enable_asserts=False,
        num_devices=8,
    )
    ins = {}
    for name, shape, dt in _IN_SPECS:
        h = nc.dram_tensor(name, list(shape), getattr(mybir.dt, dt),
                           kind="ExternalInput")
        ins[name] = h[:]
    out_h = nc.dram_tensor("out", [128, T // 4], mybir.dt.float32,
                           kind="ExternalOutput")
    build(nc, {"out": out_h[:]}, ins)
    nc.compile()
    return nc


def kernel(_trace=False, **inputs):
    from concourse.bass_utils import run_bass_kernel_spmd

    w1a, w1b, w2a, w2b, b2 = prep_weights(inputs)
    in_maps = []
    for g in range(G):
        in_maps.append(dict(xln=prep_x(g, inputs), w1a=w1a, w1b=w1b,
                            w2a=w2a, w2b=w2b))

    nc = _build_nc()
    res = run_bass_kernel_spmd(nc, in_maps, core_ids=list(range(G)),
                               trace=_trace)
    if _trace:
        print("HW exec time:", res.exec_time_ns, "ns  (mean:",
              res.mean_exec_time_ns, "ns, slowest core:",
              res.max_exec_time_core_id, ")")
        if res.instructions_and_trace:
            print("trace:", res.instructions_and_trace[1])

    attn = np.asarray(inputs["attn_bias"], np.float32)      # [G, 513, 513]
    virt = np.asarray(inputs["virt"], np.float32).reshape(H)
    outs = []
    for g, r in enumerate(res.results):
        # device rows: (g%4)*32 + h; cols: (g//4)*512 + c
        dev = r["out"].reshape(4, 32, NQ, 512).transpose(1, 2, 0, 3)
        dev = dev.reshape(32, T).reshape(32, N, N)          # [H, i, j]
        full = np.empty((H, NP1, NP1), np.float32)
        full[:, 1:, 1:] = dev + b2[:, None, None] + attn[g][None, 1:, 1:]
        full[:, 1:, 0] = attn[g][None, 1:, 0] + virt[:, None]
        full[:, 0, :] = attn[g][None, 0, :] + virt[:, None]
        edge_bias_host(g, inputs, full)
        outs.append(full)
    return np.stack(outs).astype(np.float32)
